# revision 38
# baseline (speedup 1.0000x reference)
"""GCN encoder (3-layer, PyG GCNConv normalize=False + BN eval + ReLU) on 8 trn2 cores.

V2 strategy (node/dst-sharded, graph-parallel):
  - Nodes are remapped into 8 cores x 49 tiles x 128 slots, balanced by in-degree.
    Tiles are grouped [8,8,8,8,8,8,1] and the global z-table uses a group-major
    layout so AllGathers can be chunked per group (contiguous output ranges).
  - Layer 1 z-table (z1 = x @ W1') is computed REPLICATED on every core from a
    streamed full x^T (no collective); layers 2/3 z-tables are produced
    dst-sharded and AllGathered in 7 chunks overlapped with tile compute.
  - Per dst tile: gather z[src] rows for its edges with dma_gather; the
    weighted segment-sum is a chain of 128-edge one-hot matmuls accumulating in
    PSUM. One-hot scatter matrices are generated ON-CHIP by DVE:
    onehot[lane, j] = (iota[j] == dslot[lane]) * w[lane] (two tensor_tensor ops
    per tile with broadcast views; pad lanes have dslot=-1 -> zero column).
  - BN scale is folded into the dense weights host-side; epilogue is
    add(bias) + relu (DVE), PSUM->SBUF copies on the scalar engine.
"""

import math
from dataclasses import dataclass

import ml_dtypes
import numpy as np

P = 128
HIDDEN = 256
GROUPS = (12, 12, 8, 8, 4, 2, 2, 1)  # tile groups (descending: small tail chunks)


@dataclass
class Cfg:
    n: int = 50000
    e: int = 1600000
    ncores: int = 8
    tiles: int = 49  # dst node tiles of 128 slots per core
    cin: int = 128

    @property
    def slots_per_core(self) -> int:
        return self.tiles * P

    @property
    def total_slots(self) -> int:
        return self.ncores * self.slots_per_core

    split: int = 25088  # z-table row split for int16 gather indices (set by _prep)


CFG = Cfg()

_GSTART = np.concatenate([[0], np.cumsum(GROUPS)]).astype(np.int64)  # per-core tile
_GBASE = _GSTART * (8 * P)  # table row base of each group


def _group_of_tile(t):
    return np.searchsorted(_GSTART, t, side="right") - 1


def _table_row(core, tile, slot, cfg: Cfg):
    """Group-major z-table row for (core, tile, slot). Vectorized."""
    g = _group_of_tile(tile)
    sz = np.asarray(GROUPS, dtype=np.int64)[g]
    tt = tile - _GSTART[g]
    return _GBASE[g] + core * sz * P + tt * P + slot


# ---------------------------------------------------------------------------
# Host-side preprocessing
# ---------------------------------------------------------------------------

def _balance_nodes(indeg: np.ndarray, cfg: Cfg) -> np.ndarray:
    """Assign each node a slot in [0, total_slots) so that each 128-slot tile has
    roughly equal total in-degree. Returns slot_of_node [n] (per-core-major)."""
    import heapq

    nbins = cfg.ncores * cfg.tiles
    order = np.argsort(-indeg, kind="stable")
    heap = [(0, b) for b in range(nbins)]
    heapq.heapify(heap)
    counts = np.zeros(nbins, dtype=np.int64)
    slot_of = np.empty(cfg.n, dtype=np.int64)
    for v in order:
        load, b = heapq.heappop(heap)
        slot_of[v] = b * P + counts[b]
        counts[b] += 1
        load += int(indeg[v])
        if counts[b] < P:
            heapq.heappush(heap, (load, b))
    return slot_of


def _prep(cfg: Cfg, x, edge_index, edge_attr, W1, b1, g1, beta1, m1, v1,
          W2, b2, g2, beta2, m2, v2, W3, b3):
    bf16 = ml_dtypes.bfloat16
    n, e = cfg.n, cfg.e
    src = np.asarray(edge_index[0], dtype=np.int64)
    dst = np.asarray(edge_index[1], dtype=np.int64)
    ew = np.asarray(edge_attr, dtype=np.float32).mean(axis=1)

    indeg = np.bincount(dst, minlength=n)
    slot_of = _balance_nodes(indeg, cfg)  # per-core-major slot id

    # decompose per-core-major slot -> (core, tile, slot)
    s_core = slot_of // cfg.slots_per_core
    s_rem = slot_of % cfg.slots_per_core
    s_tile = s_rem // P
    s_slot = s_rem % P
    trow_of = _table_row(s_core, s_tile, s_slot, cfg)  # group-major table row

    sslot = trow_of[src]  # source table row per edge
    d_core = s_core[dst]
    d_tile = s_tile[dst]
    d_slot = s_slot[dst]
    ebin = d_core * cfg.tiles + d_tile

    # choose the lo/hi split (int16 gather-index windows) minimizing chunks
    best = None
    for half in (20480, 24576, 25088, 28672, 32768):
        hi_cnt = np.bincount(ebin[sslot >= half], minlength=cfg.ncores * cfg.tiles)
        lo_cnt = np.bincount(ebin[sslot < half], minlength=cfg.ncores * cfg.tiles)
        c = math.ceil(lo_cnt.max() / P) + math.ceil(hi_cnt.max() / P)
        if best is None or c < best[0]:
            best = (c, half)
    cfg.split = best[1]
    is_hi = sslot >= cfg.split

    nbins = cfg.ncores * cfg.tiles
    key = ebin * 2 + is_hi.astype(np.int64)
    order = np.argsort(key, kind="stable")
    key_s = key[order]
    counts_g = np.bincount(key_s, minlength=nbins * 2)
    gstart = np.zeros(nbins * 2, dtype=np.int64)
    gstart[1:] = np.cumsum(counts_g)[:-1]
    rank = np.arange(e, dtype=np.int64) - gstart[key_s]  # rank within group

    lo_counts = counts_g[0::2].reshape(cfg.ncores, cfg.tiles)
    hi_counts = counts_g[1::2].reshape(cfg.ncores, cfg.tiles)
    ct_lo = int(math.ceil(lo_counts.max() / P))
    ct_hi = int(math.ceil(hi_counts.max() / P))
    ct = ct_lo + ct_hi

    # per-edge destination in padded chunk arrays
    e_bin = key_s // 2
    e_hi = key_s % 2
    e_core = e_bin // cfg.tiles
    e_tile = e_bin % cfg.tiles
    e_chunk = rank // P + e_hi * ct_lo
    e_lane = rank % P
    e_sslot = sslot[order] - e_hi * cfg.split  # index into z half-table
    e_dlocal = d_slot[order]  # dst slot within tile
    e_w12 = ew[order].astype(np.float32)

    # IDX[core, tile, chunk, lane] int16, pad = 0 (valid row, weight 0)
    idx = np.zeros((cfg.ncores, cfg.tiles, ct, P), dtype=np.int16)
    idx[e_core, e_tile, e_chunk, e_lane] = e_sslot.astype(np.int16)

    # dense one-hot scatter matrices for layers 1-2 (weighted), lhsT layout
    # MT[core, tile, lane(=edge, partition dim), chunk, dstslot]
    mt12f = np.zeros((cfg.ncores, cfg.tiles, P, ct, P), dtype=np.float32)
    np.add.at(mt12f, (e_core, e_tile, e_lane, e_chunk, e_dlocal), e_w12)
    mt12 = mt12f.astype(bf16)
    del mt12f
    # layers 2/3 scatter matrices are generated on-chip from dslot (+w)
    dslot_a = np.full((cfg.ncores, cfg.tiles, ct, P), -1.0, dtype=np.float32)
    dslot_a[e_core, e_tile, e_chunk, e_lane] = e_dlocal.astype(np.float32)
    dslot_sb = dslot_a.transpose(0, 3, 1, 2).reshape(
        cfg.ncores, P, cfg.tiles * ct).astype(np.float32)
    w12_a = np.zeros((cfg.ncores, cfg.tiles, ct, P), dtype=np.float32)
    w12_a[e_core, e_tile, e_chunk, e_lane] = e_w12
    wlane_sb = w12_a.transpose(0, 3, 1, 2).reshape(
        cfg.ncores, P, cfg.tiles * ct).astype(np.float32)
    iota = np.broadcast_to(np.arange(P, dtype=np.float32), (P, P)).copy().astype(bf16)

    # per-(tile, sub-gather) real index counts (pad descriptors are skipped
    # at runtime via num_idxs_reg); sub-split must match the device build
    gcnt = np.zeros((cfg.ncores, cfg.tiles, 4), dtype=np.int32)
    for half_i, (cnts, ctg) in enumerate(((lo_counts, ct_lo), (hi_counts, ct_hi))):
        a = (ctg + 1) // 2
        c1 = np.minimum(cnts, a * P)
        c2 = cnts - c1
        for sub_i, cc in enumerate((c1, c2)):
            r = np.maximum(16, ((cc + 15) // 16) * 16)
            gcnt[:, :, half_i * 2 + sub_i] = r

    # gather-call index layout: per (tile, group) block of ctg*8 columns;
    # value at (partition p, col s) = idx_linear[s*16 + p%16], replicated x8.
    idx_sb = np.zeros((cfg.ncores, P, cfg.tiles * ct * 8), dtype=np.int16)
    for g, ctg, off in ((0, ct_lo, 0), (1, ct_hi, ct_lo)):
        if ctg == 0:
            continue
        blk = idx[:, :, off:off + ctg, :].reshape(cfg.ncores, cfg.tiles, ctg * P)
        cols = blk.reshape(cfg.ncores, cfg.tiles, ctg * 8, 16)
        colbase = off * 8
        for tcol in range(ctg * 8):
            dst_col = np.arange(cfg.tiles) * (ct * 8) + colbase + tcol
            idx_sb[:, :16, dst_col] = cols[:, :, tcol, :].transpose(0, 2, 1)
    idx_sb[:, 16:, :] = np.tile(idx_sb[:, :16, :], (1, 7, 1))

    # full x in table-row order [total_slots, cin] bf16 (gather source, layer 1)
    node_of_slot = np.full(cfg.total_slots, -1, dtype=np.int64)
    node_of_slot[slot_of] = np.arange(n)  # per-core-major (for output unshard)
    node_of_trow = np.full(cfg.total_slots, -1, dtype=np.int64)
    node_of_trow[trow_of] = np.arange(n)
    xf = np.asarray(x, dtype=np.float32)
    xtab = np.zeros((cfg.total_slots, cfg.cin), dtype=np.float32)
    valid = node_of_trow >= 0
    xtab[valid] = xf[node_of_trow[valid]]
    xtab = xtab.astype(bf16)  # [total_slots, cin]

    # weights / epilogue params; fold BN scale s into the producing weights
    eps = 1e-5
    s1 = (np.asarray(g1) / np.sqrt(np.asarray(v1) + eps)).astype(np.float32)
    t1 = (np.asarray(beta1) + (np.asarray(b1) - np.asarray(m1)) * s1).astype(np.float32)
    s2 = (np.asarray(g2) / np.sqrt(np.asarray(v2) + eps)).astype(np.float32)
    t2 = (np.asarray(beta2) + (np.asarray(b2) - np.asarray(m2)) * s2).astype(np.float32)

    def rep(v):
        return np.broadcast_to(np.asarray(v, np.float32), (P, HIDDEN)).copy()

    w1t = (np.asarray(W1, np.float32).T * s1[None, :]).astype(bf16)  # [cin, 256]
    w2t = (np.asarray(W2, np.float32).T * s2[None, :]).reshape(
        2, P, HIDDEN).astype(bf16)
    w3t = np.asarray(W3, np.float32).T.reshape(2, P, HIDDEN).astype(bf16)

    in_maps = []
    for c in range(cfg.ncores):
        in_maps.append({
            "xt": xtab,
            "idx": np.ascontiguousarray(idx_sb[c]),
            "mt12": np.ascontiguousarray(mt12[c].reshape(cfg.tiles, P, ct * P)),
            "dslot": np.ascontiguousarray(dslot_sb[c]),
            "wlane": np.ascontiguousarray(wlane_sb[c]),
            "iota": iota,
            "w1t": w1t,
            "w2t": w2t,
            "w3t": w3t,
            "bi1": rep(t1), "bi2": rep(t2),
            "bi3": rep(np.asarray(b3, np.float32)),
        })
    return in_maps, node_of_slot, ct_lo, ct_hi


# ---------------------------------------------------------------------------
# Bass program
# ---------------------------------------------------------------------------

def _build(cfg: Cfg, ct_lo: int, ct_hi: int):
    import concourse.mybir as mybir
    import concourse.tile as tile
    from concourse import bacc
    from concourse.masks import make_identity

    ct = ct_lo + ct_hi
    T = cfg.tiles
    SPC = cfg.slots_per_core
    TOT = cfg.total_slots
    DT = mybir.dt
    nc = bacc.Bacc("TRN2", target_bir_lowering=False, debug=False,
                   num_devices=cfg.ncores, num_swdge_queues=4)

    xt_d = nc.declare_dram_parameter("xt", [TOT, cfg.cin], DT.bfloat16, isOutput=False)
    idx_d = nc.declare_dram_parameter("idx", [P, T * ct * 8], DT.int16, isOutput=False)
    mt12_d = nc.declare_dram_parameter("mt12", [T, P, ct * P], DT.bfloat16, isOutput=False)
    dslot_d = nc.declare_dram_parameter("dslot", [P, T * ct], DT.float32, isOutput=False)
    wlane_d = nc.declare_dram_parameter("wlane", [P, T * ct], DT.float32, isOutput=False)
    iota_d = nc.declare_dram_parameter("iota", [P, P], DT.bfloat16, isOutput=False)
    w1t_d = nc.declare_dram_parameter("w1t", [cfg.cin, HIDDEN], DT.bfloat16, isOutput=False)
    w2t_d = nc.declare_dram_parameter("w2t", [2, P, HIDDEN], DT.bfloat16, isOutput=False)
    w3t_d = nc.declare_dram_parameter("w3t", [2, P, HIDDEN], DT.bfloat16, isOutput=False)
    bi1_d = nc.declare_dram_parameter("bi1", [P, HIDDEN], DT.float32, isOutput=False)
    bi2_d = nc.declare_dram_parameter("bi2", [P, HIDDEN], DT.float32, isOutput=False)
    bi3_d = nc.declare_dram_parameter("bi3", [P, HIDDEN], DT.float32, isOutput=False)
    out_d = nc.declare_dram_parameter("out", [SPC, HIDDEN], DT.float32, isOutput=True)

    # z tables for layers 2/3 (AllGather outputs); layer 1 gathers xtab directly
    zfull = [None]
    for l in (1, 2):
        zfull.append(nc.dram_tensor(f"zfull{l}", [TOT, HIDDEN], DT.bfloat16,
                                    addr_space="Shared"))
    # per-group local z slices feeding the chunked AllGathers (layers 0,1)
    zsl = [[nc.dram_tensor(f"zsl{l}_{g}", [GROUPS[g] * P, HIDDEN], DT.bfloat16)
            for g in range(len(GROUPS))] for l in (0, 1)]
    groups_all = [list(range(cfg.ncores))]

    with tile.TileContext(nc) as tc:
        with (
            tc.tile_pool(name="const", bufs=1) as const_pool,
            tc.tile_pool(name="gpool", bufs=6) as g_pool,
            tc.tile_pool(name="mpool", bufs=3) as m_pool,
            tc.tile_pool(name="zpool", bufs=4) as z_pool,
            tc.tile_pool(name="epool", bufs=3) as e_pool,
            tc.tile_pool(name="tpool", bufs=2) as t_pool,
            tc.tile_pool(name="agg_ps", bufs=4, space="PSUM") as agg_psum,
            tc.tile_pool(name="tr_ps", bufs=2, space="PSUM") as tr_psum,
            tc.tile_pool(name="z_ps", bufs=2, space="PSUM") as z_psum,
        ):
            # persistent tiles
            idx_sb = const_pool.tile([P, T * ct * 8], DT.int16)
            nc.sync.dma_start(idx_sb[:], idx_d[:])
            dslot_sb = const_pool.tile([P, T * ct], DT.float32)
            nc.sync.dma_start(dslot_sb[:], dslot_d[:])
            wlane_sb = const_pool.tile([P, T * ct], DT.float32)
            nc.sync.dma_start(wlane_sb[:], wlane_d[:])
            iota_sb = const_pool.tile([P, P], DT.bfloat16)
            nc.sync.dma_start(iota_sb[:], iota_d[:])
            # zero-fill gather buffers once: runtime-count skip leaves stale
            # tail lanes; they multiply by zero one-hot columns and must be
            # finite (never-written SBUF could be NaN bit patterns)
            for _ in range(6):
                wlo = g_pool.tile([P, 2 * ct_lo, cfg.cin], DT.bfloat16, tag="glo")
                nc.vector.memset(wlo[:], 0.0)
                whi = g_pool.tile([P, 2 * ct_hi, cfg.cin], DT.bfloat16, tag="ghi")
                nc.vector.memset(whi[:], 0.0)
            w1t_sb = const_pool.tile([cfg.cin, HIDDEN], DT.bfloat16)
            nc.sync.dma_start(w1t_sb[:], w1t_d[:])
            w2t_sb = const_pool.tile([P, 2, HIDDEN], DT.bfloat16)
            nc.sync.dma_start(w2t_sb[:], w2t_d[:].rearrange("h p n -> p h n"))
            w3t_sb = const_pool.tile([P, 2, HIDDEN], DT.bfloat16)
            nc.sync.dma_start(w3t_sb[:], w3t_d[:].rearrange("h p n -> p h n"))
            bi_sb = []
            for i, d in enumerate((bi1_d, bi2_d, bi3_d)):
                t_ = const_pool.tile([P, HIDDEN], DT.float32, tag=f"bi{i}")
                nc.sync.dma_start(t_[:], d[:])
                bi_sb.append(t_)
            ident = const_pool.tile([P, P], DT.bfloat16)
            make_identity(nc, ident[:])

            # ---------------- 3 conv layers ----------------
            # Software-pipelined: iteration u issues gather+one-hot for tile u
            # and compute (matmuls/epilogue) for tile u-1, so DVE's in-order
            # stream pre-generates one-hots during gather waits.
            LAG = 2  # tiles of issue-ahead (gather+one-hot) vs compute
            for l in range(3):
                zf = zfull[l] if l > 0 else xt_d
                width = HIDDEN if l > 0 else cfg.cin
                pend = []  # [(t, glo, ghi, oh), ...] awaiting compute
                for u in range(T + LAG):
                    if u < T:
                        t = u
                        # split each tile's gathers across all 4 SWDGE rings
                        # (2 sub-gathers per half) to cut per-tile drain latency
                        if l > 0:
                            glo = g_pool.tile([P, ct_lo, HIDDEN], DT.bfloat16, tag="glo")
                            ghi = g_pool.tile([P, ct_hi, HIDDEN], DT.bfloat16, tag="ghi")
                        else:
                            # same byte-size buffers viewed as [P, 2*ctg, cin];
                            # only the first ctg chunks are gathered/used
                            glo = g_pool.tile([P, 2 * ct_lo, cfg.cin], DT.bfloat16, tag="glo")
                            ghi = g_pool.tile([P, 2 * ct_hi, cfg.cin], DT.bfloat16, tag="ghi")
                        qn = 0
                        for buf, ctg, off in ((glo, ct_lo, 0), (ghi, ct_hi, ct_lo)):
                            a = (ctg + 1) // 2
                            for si, (c0, c1) in enumerate(((0, a), (a, ctg))):
                                nsub = c1 - c0
                                if nsub == 0:
                                    continue
                                cb = t * ct * 8 + (off + c0) * 8
                                nc.gpsimd.dma_gather(
                                    buf[:, c0:c1, :],
                                    zf[0:cfg.split, :] if off == 0 else zf[cfg.split:, :],
                                    idx_sb[:, cb: cb + nsub * 8],
                                    nsub * P, nsub * P, width,
                                    single_packet=False, queue_num=qn % 4)
                                qn += 1
                        # scatter matrix: layer 1 streamed from DRAM (DMA-light
                        # layer); layers 2/3 generated on-chip by DVE
                        oh = m_pool.tile([P, ct * P], DT.bfloat16)
                        if l == 0:
                            nc.sync.dma_start(oh[:], mt12_d[t])
                        else:
                            for k in range(ct):
                                col = t * ct + k
                                if l == 1:
                                    nc.vector.tensor_scalar(
                                        oh[:, k * P:(k + 1) * P], iota_sb[:],
                                        dslot_sb[:, col:col + 1],
                                        wlane_sb[:, col:col + 1],
                                        op0=mybir.AluOpType.is_equal,
                                        op1=mybir.AluOpType.mult)
                                else:
                                    nc.vector.tensor_scalar(
                                        oh[:, k * P:(k + 1) * P], iota_sb[:],
                                        dslot_sb[:, col:col + 1], None,
                                        op0=mybir.AluOpType.is_equal)
                        pend.append((t, glo, ghi, oh))

                    if u < LAG:
                        continue
                    t, glo, ghi, oh = pend.pop(0)
                    g = int(_group_of_tile(t))
                    tt_local = t - int(_GSTART[g])

                    if l == 0:
                        # aggregate in x-space [128 dst, cin], then
                        # agg1 = (agg_x) @ W1'  (needs agg_x^T as lhsT)
                        psxt = agg_psum.tile([P, HIDDEN], DT.float32, tag="ps")
                        psx = psxt[:, 0:cfg.cin]
                        for k in range(ct):
                            gsrc = glo[:, k, :] if k < ct_lo else ghi[:, k - ct_lo, :]
                            nc.tensor.matmul(psx, oh[:, k * P:(k + 1) * P], gsrc,
                                             start=(k == 0), stop=(k == ct - 1))
                        psx_sb = z_pool.tile([P, cfg.cin], DT.bfloat16, tag="psx_sb")
                        nc.scalar.copy(psx_sb[:], psx)
                        tpx = tr_psum.tile([P, P], DT.bfloat16, tag="tp")
                        nc.tensor.transpose(tpx[:], psx_sb[:], ident[:])
                        ttx = t_pool.tile([P, P], DT.bfloat16, tag="ttx")
                        nc.scalar.copy(ttx[:], tpx[:])
                        ps = agg_psum.tile([P, HIDDEN], DT.float32)
                        nc.tensor.matmul(ps[:], ttx[:], w1t_sb[:],
                                         start=True, stop=True)
                    else:
                        ps = agg_psum.tile([P, HIDDEN], DT.float32)
                        for k in range(ct):
                            gsrc = glo[:, k, :] if k < ct_lo else ghi[:, k - ct_lo, :]
                            nc.tensor.matmul(ps[:], oh[:, k * P:(k + 1) * P], gsrc,
                                             start=(k == 0), stop=(k == ct - 1))

                    if l < 2:
                        tmp = e_pool.tile([P, HIDDEN], DT.float32)
                        nc.vector.tensor_tensor(
                            out=tmp[:], in0=ps[:], in1=bi_sb[l][:],
                            op=mybir.AluOpType.add)
                        relu = z_pool.tile([P, HIDDEN], DT.bfloat16, tag="relu")
                        nc.vector.tensor_scalar_max(relu[:], tmp[:], 0.0)
                        # z_{l+1} slice = relu @ W_{l+1}' (needs relu^T tiles)
                        wnext = w2t_sb if l == 0 else w3t_sb
                        zps = z_psum.tile([P, HIDDEN], DT.float32, tag="zps")
                        for h in range(2):
                            tp = tr_psum.tile([P, P], DT.bfloat16)
                            nc.tensor.transpose(
                                tp[:], relu[:, h * P:(h + 1) * P], ident[:])
                            tt = t_pool.tile([P, P], DT.bfloat16)
                            nc.scalar.copy(tt[:], tp[:])
                            nc.tensor.matmul(zps[:], tt[:], wnext[:, h, :],
                                             start=(h == 0), stop=(h == 1))
                        zn = z_pool.tile([P, HIDDEN], DT.bfloat16, tag="zn")
                        nc.scalar.copy(zn[:], zps[:])
                        nc.sync.dma_start(
                            zsl[l][g][tt_local * P:(tt_local + 1) * P, :], zn[:])
                        if t == int(_GSTART[g + 1]) - 1:
                            # group complete -> AllGather this chunk
                            base = int(_GBASE[g])
                            span = cfg.ncores * GROUPS[g] * P
                            nc.gpsimd.collective_compute(
                                "AllGather", mybir.AluOpType.bypass,
                                replica_groups=groups_all,
                                ins=[zsl[l][g][:]],
                                outs=[zfull[l + 1][base:base + span, :]])
                    else:
                        ot = e_pool.tile([P, HIDDEN], DT.float32, tag="out")
                        nc.vector.tensor_tensor(
                            out=ot[:], in0=ps[:], in1=bi_sb[2][:],
                            op=mybir.AluOpType.add)
                        nc.sync.dma_start(out_d[t * P:(t + 1) * P, :], ot[:])
    nc.compile()
    return nc


# ---------------------------------------------------------------------------
# Entry point
# ---------------------------------------------------------------------------

LAST_RESULTS = None  # BassKernelResults of the most recent _run (for profiling)


def _run(cfg: Cfg, inputs: dict, trace: bool = False,
         trace_cores=None) -> np.ndarray:
    global LAST_RESULTS
    from concourse.bass_utils import run_bass_kernel_spmd

    in_maps, node_of_slot, ct_lo, ct_hi = _prep(cfg, **inputs)
    nc = _build(cfg, ct_lo, ct_hi)
    kr = run_bass_kernel_spmd(nc, in_maps, list(range(cfg.ncores)), trace=trace,
                              trace_cores=trace_cores)
    LAST_RESULTS = kr
    res = kr.results
    out = np.empty((cfg.n, HIDDEN), dtype=np.float32)
    full = np.concatenate([res[c]["out"] for c in range(cfg.ncores)], axis=0)
    valid = node_of_slot >= 0
    out[node_of_slot[valid]] = full[valid]
    return out


def kernel(**inputs) -> np.ndarray:
    return _run(CFG, inputs)
